# revision 60
# baseline (speedup 1.0000x reference)
"""Conv2d(128->256, 3x3, pad=1) over a 256x256 image, sharded across 8 trn2 cores.

Strategy: 1-D Winograd F(2,3) along Y, all-fp16.
---------------------------------------------------
x: (C_in=128, H=256, W=256) f32, weight: (256, 128, 3, 3), bias: (256,1,1).
C_in == 128 maps exactly onto the SBUF partition (contraction) dim.

The 3x3 conv is decomposed as 3 kx taps x a 3-tap FIR in y. The y FIR is
computed with Winograd F(2,3): per output row PAIR (ty), the host builds 4
transformed row streams from padded input rows d_r = xpad[2ty + r]:

    v0 = d0 - d2,  v1 = d1 + d2,  v2 = d2 - d1,  v3 = d1 - d3

and 4 transformed weight sets per kx (g_r = w[:, :, ky=r, kx]):

    w~0 = g0, w~1 = (g0+g1+g2)/2, w~2 = (g0-g1+g2)/2, w~3 = -g2  (NEGATED)

On device, 4 plane matmuls per (sub-unit = 2 ty x one out-channel half),
each accumulating 3 kx taps (N=512):

    m_p = sum_kx  w~_p[kx].T @ v_p[ty0:ty0+2, kx:kx+W]

    out[2ty]   = m0 + m1 + m2 + bias           (even rows)
    out[2ty+1] = m1 - m2 + m3' + bias          (odd rows; m3' = -g2 conv)

That is 12 N=512 matmuls per 2ty-half where direct conv needs 18: a 33%
reduction in PE cycles, with NO fp8 anywhere (the old kernel ran 2 of 9
taps in fp8e4m3 at 1.74e-2 rel err; this is ~7e-4).

The inverse transform must combine PSUM planes post-PE (that sharing of
m1/m2 between even and odd rows IS the Winograd saving) and is budgeted
across the two fused-op engines (measured [128,512] op costs: ACT 687ns,
DVE-with-PSUM-operand 751ns, DVE 16-bit SBUF-only ~480ns):

  ScalarE: s1 = m1 + bias (fp16), s2p = +m2, s2n = -m2      3 ops ~2.0us
  VectorE: X = [m0|m3'] + bcast(s1)    (one op, stride-0 broadcast AP)
           OUT = X + [s2p|s2n] -> fp16 out rows             2 ops ~2.0us

against a 2.6us matmul span per sub-unit. PSUM planes pack pairwise into
[128, 2, 512] tiles ([m0|m3'], [m1|m2]) = 2 banks each; ring of 2
sub-units fills all 8 banks. Walrus rejects scalar_tensor_tensor with two
PSUM operands and ANY gpsimd tensor op, hence this exact split.

Sharding: H split across 8 cores (32 output rows = 16 ty pairs each); the
y halo is absorbed into the host transform (v rows never cross units).

DMA: one sync-engine queue carries all inputs in strict need-order. The
startup blob fuses v_ty0 + all half-0 weights + the fp32 bias (raw bytes
in fp16 cols) into ONE transfer: it alone gates the first real matmul.
Dep-free dummy matmuls bridge engine-boot (~6.6us) to first-data and lift
the PE HAM clock gate out of its cold 1.2 GHz state. Output tiles
([p, half, row-interleaved, x] so one DMA per 2ty unit moves both halves)
alternate sync/scalar trigger engines.
"""

import numpy as np

import concourse.bass as bass
import concourse.tile as tile
from concourse import bacc, mybir
from concourse import bass_utils

N_CORES = 8
C_IN, C_OUT, KH, KW = 128, 256, 3, 3
H, W = 256, 256
H_S = H // N_CORES            # 32 output rows per core
TY_S = H_S // 2               # 16 winograd row-pair units per core
WP = W + 2                    # padded row width: 258
N_HALF = C_OUT // 128         # 2 output-channel halves

F32 = mybir.dt.float32
F16 = mybir.dt.float16

# startup: va = [v_p1 ty0 | 4 bias cols] lands first (gates MM #1), then
# per-plane weight/v transfers in strict MM need order p1, p2, p0, p3
W0 = 3 * 128                  # per-plane weight cols
VA_C = WP + 4                 # 262
WH0_IDX = {1: 0, 2: 1, 0: 2, 3: 3}
VB_IDX = {2: 0, 0: 1, 3: 2}   # row order of the vb / wb2-style tensors
# units in ty pairs: first two 1-ty (fast start), last two 1-ty (short tail)
UNITS = [1, 1, 2, 2, 2, 2, 2, 2, 1, 1]
assert sum(UNITS) == TY_S
# v DMA groups for ty 1..15 (ty0 rides the blob)
V_GROUPS = [1, 2, 4, 8]
assert sum(V_GROUPS) == TY_S - 1

WARMUP_N512 = 8

# Set by test harness: TRACE=True makes the next kernel() call capture an
# NTFF profile; the BassKernelResults lands in LAST_RESULT.
TRACE = False
TRACE_KW = {}
LAST_RESULT = None

_NC_CACHE = {}


def _build():
    nc = bacc.Bacc(
        "TRN2",
        target_bir_lowering=False,
        debug=False,
        enable_asserts=False,
        num_devices=N_CORES,
    )
    va_d = nc.dram_tensor("va", [C_IN, 1, VA_C], F16, kind="ExternalInput").ap()
    vb_d = nc.dram_tensor("vb", [C_IN, 3, WP], F16, kind="ExternalInput").ap()
    wa_d = nc.dram_tensor("wa", [C_IN, 1, W0], F16, kind="ExternalInput").ap()
    wb1_d = nc.dram_tensor("wb1", [C_IN, 1, W0], F16, kind="ExternalInput").ap()
    wb2_d = nc.dram_tensor("wb2", [C_IN, 2, W0], F16, kind="ExternalInput").ap()
    wh1a_d = nc.dram_tensor("wh1a", [C_IN, 1, W0], F16, kind="ExternalInput").ap()
    wh1b_d = nc.dram_tensor("wh1b", [C_IN, 3, W0], F16, kind="ExternalInput").ap()
    # negated-m2 weights for the last unit's direct-accumulated odd rows
    wn_d = nc.dram_tensor("wn", [C_IN, 2, W0], F16, kind="ExternalInput").ap()
    v_d = nc.dram_tensor(
        "v", [C_IN, TY_S - 1, 4, WP], F16, kind="ExternalInput"
    ).ap()
    # output laid out [p, half, even/odd, ty, x] (channel h*128+p at [p, h];
    # row 2*ty+eo at [eo, ty]); host deinterleaves rows during the gather
    o_d = nc.dram_tensor(
        "out", [128, N_HALF, 2, TY_S, W], F16, kind="ExternalOutput"
    ).ap()

    with tile.TileContext(nc) as tc:
        with (
            tc.tile_pool(name="vin", bufs=1) as vpool,
            tc.tile_pool(name="wts", bufs=1) as wpool,
            tc.tile_pool(name="acc", bufs=4, space="PSUM") as ppool,
            tc.tile_pool(name="scr", bufs=8) as spool,
            tc.tile_pool(name="outs", bufs=6) as opool,
        ):
            # PE warmup: dep-free garbage through the PE into a dead tile
            warm_sb = nc.alloc_sbuf_tensor("warm_src", [128, 512], F16).ap()
            warm_ps = ppool.tile([128, 2, 512], F32, tag="ps", name="ps")
            for _ in range(WARMUP_N512):
                nc.tensor.matmul(warm_ps[:, 0, :], warm_sb[:, :128], warm_sb[:])
            # ACT warmup: absorb the ~1.3us one-time ACT_TABLE_LOAD during
            # boot so the first real finish op isn't delayed by it
            warm_act = nc.alloc_sbuf_tensor("warm_act", [128, 16], F16).ap()
            nc.scalar.activation(
                warm_act[:], warm_sb[:, :16],
                mybir.ActivationFunctionType.Identity, bias=0.0, scale=1.0,
            )

            va_sb = wpool.tile([128, 1, VA_C], F16, tag="va", name="va")
            vb_sb = wpool.tile([128, 3, WP], F16, tag="vb", name="vb")
            wa_sb = wpool.tile([128, 1, W0], F16, tag="wa", name="wa")
            wb1_sb = wpool.tile([128, 1, W0], F16, tag="wb1", name="wb1")
            wb2_sb = wpool.tile([128, 2, W0], F16, tag="wb2", name="wb2")
            wh1a_sb = wpool.tile([128, 1, W0], F16, tag="wh1a", name="wh1a")
            wh1b_sb = wpool.tile([128, 3, W0], F16, tag="wh1b", name="wh1b")
            vg_sb = [
                vpool.tile([128, g, 4, WP], F16, tag=f"vg{i}", name=f"vg{i}")
                for i, g in enumerate(V_GROUPS)
            ]
            vg_t0 = [1 + sum(V_GROUPS[:i]) for i in range(len(V_GROUPS))]

            # single queue, strict need-order
            wn_sb = wpool.tile([128, 2, W0], F16, tag="wn", name="wn")
            # wn rides the scalar HWDGE queue at boot: it is only needed at
            # the very end, and the transfer warms up queue 10 so the tail
            # output DMAs routed there don't pay its ~2us cold start
            nc.scalar.dma_start(wn_sb[:], wn_d[:])
            # sync queue, strict need-order; sub-units consume in the
            # interleaved order u0h0, u1h0, u0h1, u1h1, u2h0, ... so the h1
            # weights and early v groups have extra time to land
            nc.sync.dma_start(va_sb[:], va_d[:])
            nc.sync.dma_start(wa_sb[:], wa_d[:])
            nc.sync.dma_start(vb_sb[:], vb_d[:])
            nc.sync.dma_start(wb1_sb[:], wb1_d[:])
            nc.sync.dma_start(wb2_sb[:], wb2_d[:])
            vg_dmas = [
                (vg_sb[i], v_d[:, vg_t0[i] - 1 : vg_t0[i] - 1 + g, :, :])
                for i, g in enumerate(V_GROUPS)
            ]
            nc.sync.dma_start(*vg_dmas[0])
            nc.sync.dma_start(wh1a_sb[:], wh1a_d[:])
            nc.sync.dma_start(wh1b_sb[:], wh1b_d[:])
            for t in vg_dmas[1:]:
                nc.sync.dma_start(*t)

            # fp32 bias bytes for half h live in va cols 258+2h : 260+2h
            bias_ap = [
                va_sb[:, 0, WP + 2 * h : WP + 2 * h + 2].bitcast(F32)
                for h in range(N_HALF)
            ]

            def v_ap(ty, nty, p, kx):
                """moving operand [128, nty, W] for ty..ty+nty-1, stream p"""
                if ty == 0:
                    assert nty == 1
                    if p == 1:
                        return va_sb[:, 0, kx : kx + W]
                    return vb_sb[:, VB_IDX[p], kx : kx + W]
                for i in reversed(range(len(V_GROUPS))):
                    if ty >= vg_t0[i]:
                        t = ty - vg_t0[i]
                        assert t + nty <= V_GROUPS[i]
                        return vg_sb[i][:, t : t + nty, p, kx : kx + W]
                raise AssertionError(ty)

            def w_ap(h, p, kx):
                if h == 0:
                    if p == 1:
                        wsb, row = wa_sb, 0
                    elif p == 2:
                        wsb, row = wb1_sb, 0
                    else:
                        wsb, row = wb2_sb, WH0_IDX[p] - 2
                else:
                    wsb, row = (
                        (wh1a_sb, 0) if p == 1 else (wh1b_sb, WH0_IDX[p] - 1)
                    )
                return wsb[:, row, kx * 128 : kx * 128 + 128]

            def bcast2(ap):
                """[128, N] AP -> [128, 2, N] with a stride-0 middle dim"""
                return bass.AP(ap.tensor, ap.offset, [ap.ap[0], [0, 2], ap.ap[1]])

            add = mybir.AluOpType.add
            ident = mybir.ActivationFunctionType.Identity
            UT0 = [sum(UNITS[:i]) for i in range(len(UNITS))]
            sched = [(u, h) for u in range(len(UNITS)) for h in range(2)]
            for u, h in sched:
                nty = UNITS[u]
                ty0 = UT0[u]
                n = nty * W
                if u == len(UNITS) - 1:
                    # last unit: accumulate each output row COMPLETELY in
                    # PSUM (9 MMs per row, odd rows via negated-m2 weights)
                    # so the tail chain is just ACT + DMA - no VectorE drain
                    pl = ppool.tile([128, 2, n], F32, tag="ps", name="ps")
                    for j, plist in ((0, (1, 2, 0)), (1, (1, 5, 3))):
                        for idx, p in enumerate(plist):
                            for kx in range(KW):
                                if p == 5:
                                    lhsT = wn_sb[:, h, kx * 128 : kx * 128 + 128]
                                    vp = 2
                                else:
                                    lhsT = w_ap(h, p, kx)
                                    vp = p
                                nc.tensor.matmul(
                                    pl[:, j, :],
                                    lhsT,
                                    v_ap(ty0, nty, vp, kx),
                                    start=(idx == 0 and kx == 0),
                                    stop=(idx == 2 and kx == KW - 1),
                                )
                    # even rows on ACT, odd on V (parallel tail chains);
                    # each row DMAs out the moment its plane is done
                    oh = opool.tile([128, 2, n], F16, tag="ot", name="ot")
                    nc.scalar.activation(
                        oh[:, 0, :],
                        pl[:, 0, :],
                        ident,
                        bias=bias_ap[h][:, 0:1],
                        scale=1.0,
                    )
                    nc.vector.tensor_scalar(
                        out=oh[:, 1, :],
                        in0=pl[:, 1, :],
                        scalar1=1.0,
                        scalar2=bias_ap[h][:, 0:1],
                        op0=mybir.AluOpType.mult,
                        op1=add,
                    )
                    for j in range(2):
                        # h0's rows on the scalar queue, h1 (critical) on
                        # sync: the tail wire drains on both in parallel
                        eng = nc.scalar if h == 0 else nc.sync
                        eng.dma_start(
                            o_d[:, h, j, ty0 : ty0 + nty, :], oh[:, j, :]
                        )
                    continue
                # plane pairs: ta = [m1 | m2], tb = [m0 | m3']
                ta = ppool.tile([128, 2, n], F32, tag="ps", name="ps")
                tb = ppool.tile([128, 2, n], F32, tag="ps", name="ps")
                for mt, planes in ((ta, (1, 2)), (tb, (0, 3))):
                    for j, p in enumerate(planes):
                        for kx in range(KW):
                            nc.tensor.matmul(
                                mt[:, j, :],
                                w_ap(h, p, kx),
                                v_ap(ty0, nty, p, kx),
                                start=(kx == 0),
                                stop=(kx == KW - 1),
                            )
                # ScalarE: stage m1+bias and +-m2 as fp16
                s1 = spool.tile([128, n], F16, tag="s1", name="s1")
                s2 = spool.tile([128, 2, n], F16, tag="s2", name="s2")
                nc.scalar.activation(
                    s1[:],
                    ta[:, 0, :],
                    ident,
                    bias=bias_ap[h][:, 0:1],
                    scale=1.0,
                )
                nc.scalar.activation(
                    s2[:, 0, :], ta[:, 1, :], ident, bias=0.0, scale=1.0
                )
                nc.scalar.activation(
                    s2[:, 1, :], ta[:, 1, :], ident, bias=0.0, scale=-1.0
                )
                # VectorE: X = [m0|m3'] + [s1|s1] (stride-0 broadcast),
                # OUT = X + [s2|-s2] -> [even|odd] rows contiguous
                xp = spool.tile([128, 2, n], F16, tag="xp", name="xp")
                nc.vector.scalar_tensor_tensor(
                    out=xp[:],
                    in0=tb[:],
                    scalar=0.0,
                    in1=bcast2(s1[:]),
                    op0=add,
                    op1=add,
                )
                oh = opool.tile([128, 2, n], F16, tag="ot", name="ot")
                nc.vector.scalar_tensor_tensor(
                    out=oh[:],
                    in0=xp[:],
                    scalar=0.0,
                    in1=s2[:],
                    op0=add,
                    op1=add,
                )
                eng = (
                    nc.scalar if (u == len(UNITS) - 2 and h == 1) else nc.sync
                )
                eng.dma_start(o_d[:, h, :, ty0 : ty0 + nty, :], oh[:])
    nc.compile()
    return nc


def kernel(x, weight, bias):
    global LAST_RESULT
    if "nc" not in _NC_CACHE:
        _NC_CACHE["nc"] = _build()
    nc = _NC_CACHE["nc"]

    x = np.ascontiguousarray(np.asarray(x, dtype=np.float32))
    weight = np.asarray(weight, dtype=np.float32)
    bias = np.asarray(bias, dtype=np.float32)

    # zero-padded image; host computes the y-direction Winograd transform
    xpad = np.zeros((C_IN, H + 2, WP), dtype=np.float32)
    xpad[:, 1 : H + 1, 1 : W + 1] = x
    TY = H // 2
    r = 2 * np.arange(TY)
    v_full = np.empty((4, C_IN, TY, WP), dtype=np.float16)
    v_full[0] = xpad[:, r] - xpad[:, r + 2]
    v_full[1] = xpad[:, r + 1] + xpad[:, r + 2]
    v_full[2] = xpad[:, r + 2] - xpad[:, r + 1]
    v_full[3] = xpad[:, r + 1] - xpad[:, r + 3]

    # winograd weights per (p, kx): [c, o] fp16; w~3 NEGATED (odd row adds)
    wT = weight.transpose(1, 2, 3, 0)  # [c, ky, kx, o]
    wt = np.empty((4, KW, C_IN, C_OUT), dtype=np.float16)
    wt[0] = wT[:, 0].transpose(1, 0, 2)
    wt[1] = ((wT[:, 0] + wT[:, 1] + wT[:, 2]) / 2).transpose(1, 0, 2)
    wt[2] = ((wT[:, 0] - wT[:, 1] + wT[:, 2]) / 2).transpose(1, 0, 2)
    wt[3] = (-wT[:, 2]).transpose(1, 0, 2)

    # b[p, h] = bias[h*128 + p] in fp32, as raw bytes in fp16-typed cols
    bh = np.ascontiguousarray(bias.reshape(N_HALF, 128).T.astype(np.float32))
    bhv = bh.view(np.float16)  # [128, 4]

    def wpack(p, o0):
        return wt[p][:, :, o0 : o0 + 128].transpose(1, 0, 2).reshape(C_IN, KW * 128)

    wa = wpack(1, 0).reshape(C_IN, 1, W0)
    wb1 = wpack(2, 0).reshape(C_IN, 1, W0)
    wb2 = np.stack([wpack(p, 0) for p in (0, 3)], axis=1)
    wh1a = wpack(1, 128).reshape(C_IN, 1, W0)
    wh1b = np.stack([wpack(p, 128) for p in (2, 0, 3)], axis=1)
    wn = np.stack([-wpack(2, 0), -wpack(2, 128)], axis=1)

    in_maps = []
    for c in range(N_CORES):
        t0 = c * TY_S
        va = np.zeros((C_IN, 1, VA_C), dtype=np.float16)
        va[:, 0, :WP] = v_full[1][:, t0, :]
        va[:, 0, WP:] = bhv
        vb = np.ascontiguousarray(
            np.stack([v_full[p][:, t0, :] for p in (2, 0, 3)], axis=1)
        )
        vc = np.ascontiguousarray(
            v_full[:, :, t0 + 1 : t0 + TY_S, :].transpose(1, 2, 0, 3)
        )
        in_maps.append(
            {
                "va": va,
                "vb": vb,
                "wa": wa,
                "wb1": wb1,
                "wb2": wb2,
                "wh1a": wh1a,
                "wh1b": wh1b,
                "wn": wn,
                "v": vc,
            }
        )

    kw = dict(TRACE_KW)
    if TRACE:
        kw.setdefault("trace", True)
        kw.setdefault("trace_cores", [0])
    res = bass_utils.run_bass_kernel_spmd(
        nc, in_maps, core_ids=list(range(N_CORES)), **kw
    )
    LAST_RESULT = res

    out = np.empty((C_OUT, H, W), dtype=np.float32)
    for c in range(N_CORES):
        # device layout [p, half, eo, ty, x]: channel h*128+p, row 2*ty+eo
        arr = res.results[c]["out"].astype(np.float32)
        out[:, c * H_S : (c + 1) * H_S, :] = arr.transpose(1, 0, 3, 2, 4).reshape(
            C_OUT, H_S, W
        )
    return out


# revision 61
# speedup vs baseline: 1.0901x; 1.0901x over previous
"""Conv2d(128->256, 3x3, pad=1) over a 256x256 image, sharded across 8 trn2 cores.

Strategy: 1-D Winograd F(2,3) along Y, all-fp16.
---------------------------------------------------
x: (C_in=128, H=256, W=256) f32, weight: (256, 128, 3, 3), bias: (256,1,1).
C_in == 128 maps exactly onto the SBUF partition (contraction) dim.

The 3x3 conv is decomposed as 3 kx taps x a 3-tap FIR in y. The y FIR is
computed with Winograd F(2,3): per output row PAIR (ty), the host builds 4
transformed row streams from padded input rows d_r = xpad[2ty + r]:

    v0 = d0 - d2,  v1 = d1 + d2,  v2 = d2 - d1,  v3 = d1 - d3

and 4 transformed weight sets per kx (g_r = w[:, :, ky=r, kx]):

    w~0 = g0, w~1 = (g0+g1+g2)/2, w~2 = (g0-g1+g2)/2, w~3 = -g2  (NEGATED)

On device, 4 plane matmuls per (sub-unit = 2 ty x one out-channel half),
each accumulating 3 kx taps (N=512):

    m_p = sum_kx  w~_p[kx].T @ v_p[ty0:ty0+2, kx:kx+W]

    out[2ty]   = m0 + m1 + m2 + bias           (even rows)
    out[2ty+1] = m1 - m2 + m3' + bias          (odd rows; m3' = -g2 conv)

That is 12 N=512 matmuls per 2ty-half where direct conv needs 18: a 33%
reduction in PE cycles, with NO fp8 anywhere (the old kernel ran 2 of 9
taps in fp8e4m3 at 1.74e-2 rel err; this is ~7e-4).

The inverse transform must combine PSUM planes post-PE (that sharing of
m1/m2 between even and odd rows IS the Winograd saving) and is budgeted
across the two fused-op engines (measured [128,512] op costs: ACT 687ns,
DVE-with-PSUM-operand 751ns, DVE 16-bit SBUF-only ~480ns):

  ScalarE: s1 = m1 + bias (fp16), s2p = +m2, s2n = -m2      3 ops ~2.0us
  VectorE: X = [m0|m3'] + bcast(s1)    (one op, stride-0 broadcast AP)
           OUT = X + [s2p|s2n] -> fp16 out rows             2 ops ~2.0us

against a 2.6us matmul span per sub-unit. PSUM planes pack pairwise into
[128, 2, 512] tiles ([m0|m3'], [m1|m2]) = 2 banks each; ring of 2
sub-units fills all 8 banks. Walrus rejects scalar_tensor_tensor with two
PSUM operands and ANY gpsimd tensor op, hence this exact split.

Sharding: H split across 8 cores (32 output rows = 16 ty pairs each); the
y halo is absorbed into the host transform (v rows never cross units).

DMA: one sync-engine queue carries all inputs in strict need-order. The
startup blob fuses v_ty0 + all half-0 weights + the fp32 bias (raw bytes
in fp16 cols) into ONE transfer: it alone gates the first real matmul.
Dep-free dummy matmuls bridge engine-boot (~6.6us) to first-data and lift
the PE HAM clock gate out of its cold 1.2 GHz state. Output tiles
([p, half, row-interleaved, x] so one DMA per 2ty unit moves both halves)
alternate sync/scalar trigger engines.
"""

import numpy as np

import concourse.bass as bass
import concourse.tile as tile
from concourse import bacc, mybir
from concourse import bass_utils

N_CORES = 8
C_IN, C_OUT, KH, KW = 128, 256, 3, 3
H, W = 256, 256
H_S = H // N_CORES            # 32 output rows per core
TY_S = H_S // 2               # 16 winograd row-pair units per core
WP = W + 2                    # padded row width: 258
N_HALF = C_OUT // 128         # 2 output-channel halves

F32 = mybir.dt.float32
F16 = mybir.dt.float16

# startup mega-blob row p = [v_p ty0 (258) | h0 weights w~_p[kx0..2] (384) |
# 2 bias cols]: ONE transfer unblocks the whole first unit atomically, so
# the early stream has no per-transfer arrival gaps (HAM window stays busy)
W0 = 3 * 128                  # per-plane weight cols
BLOB_C = WP + W0 + 2          # 644
WH0_IDX = {1: 0, 2: 1, 0: 2, 3: 3}
# units in ty pairs: first two 1-ty (fast start), last two 1-ty (short tail)
UNITS = [1, 1, 2, 2, 2, 2, 2, 2, 1, 1]
assert sum(UNITS) == TY_S
# v DMA groups for ty 1..15 (ty0 rides the blob)
V_GROUPS = [1, 2, 4, 8]
assert sum(V_GROUPS) == TY_S - 1

WARMUP_N512 = 10

# Set by test harness: TRACE=True makes the next kernel() call capture an
# NTFF profile; the BassKernelResults lands in LAST_RESULT.
TRACE = False
TRACE_KW = {}
LAST_RESULT = None

_NC_CACHE = {}


def _build():
    nc = bacc.Bacc(
        "TRN2",
        target_bir_lowering=False,
        debug=False,
        enable_asserts=False,
        num_devices=N_CORES,
    )
    vw_d = nc.dram_tensor("vw", [C_IN, 4, BLOB_C], F16, kind="ExternalInput").ap()
    wh1a_d = nc.dram_tensor("wh1a", [C_IN, 1, W0], F16, kind="ExternalInput").ap()
    wh1b_d = nc.dram_tensor("wh1b", [C_IN, 3, W0], F16, kind="ExternalInput").ap()
    # negated-m2 weights for the last unit's direct-accumulated odd rows
    wn_d = nc.dram_tensor("wn", [C_IN, 2, W0], F16, kind="ExternalInput").ap()
    v_d = nc.dram_tensor(
        "v", [C_IN, TY_S - 1, 4, WP], F16, kind="ExternalInput"
    ).ap()
    # output laid out [p, half, even/odd, ty, x] (channel h*128+p at [p, h];
    # row 2*ty+eo at [eo, ty]); host deinterleaves rows during the gather
    o_d = nc.dram_tensor(
        "out", [128, N_HALF, 2, TY_S, W], F16, kind="ExternalOutput"
    ).ap()

    with tile.TileContext(nc) as tc:
        with (
            tc.tile_pool(name="vin", bufs=1) as vpool,
            tc.tile_pool(name="wts", bufs=1) as wpool,
            tc.tile_pool(name="acc", bufs=4, space="PSUM") as ppool,
            tc.tile_pool(name="scr", bufs=8) as spool,
            tc.tile_pool(name="outs", bufs=6) as opool,
        ):
            # PE warmup: dep-free garbage through the PE into a dead tile
            warm_sb = nc.alloc_sbuf_tensor("warm_src", [128, 512], F16).ap()
            warm_ps = ppool.tile([128, 2, 512], F32, tag="ps", name="ps")
            for _ in range(WARMUP_N512):
                nc.tensor.matmul(warm_ps[:, 0, :], warm_sb[:, :128], warm_sb[:])
            # ACT warmup: absorb the ~1.3us one-time ACT_TABLE_LOAD during
            # boot so the first real finish op isn't delayed by it
            warm_act = nc.alloc_sbuf_tensor("warm_act", [128, 16], F16).ap()
            nc.scalar.activation(
                warm_act[:], warm_sb[:, :16],
                mybir.ActivationFunctionType.Identity, bias=0.0, scale=1.0,
            )

            blob_sb = wpool.tile([128, 4, BLOB_C], F16, tag="vw", name="vw")
            wh1a_sb = wpool.tile([128, 1, W0], F16, tag="wh1a", name="wh1a")
            wh1b_sb = wpool.tile([128, 3, W0], F16, tag="wh1b", name="wh1b")
            vg_sb = [
                vpool.tile([128, g, 4, WP], F16, tag=f"vg{i}", name=f"vg{i}")
                for i, g in enumerate(V_GROUPS)
            ]
            vg_t0 = [1 + sum(V_GROUPS[:i]) for i in range(len(V_GROUPS))]

            # single queue, strict need-order
            wn_sb = wpool.tile([128, 2, W0], F16, tag="wn", name="wn")
            # wn rides the scalar HWDGE queue at boot: it is only needed at
            # the very end, and the transfer warms up queue 10 so the tail
            # output DMAs routed there don't pay its ~2us cold start
            nc.scalar.dma_start(wn_sb[:], wn_d[:])
            # sync queue, strict need-order; sub-units consume in the
            # interleaved order u0h0, u1h0, u0h1, u1h1, u2h0, ... so the h1
            # weights and early v groups have extra time to land
            nc.sync.dma_start(blob_sb[:], vw_d[:])
            nc.sync.dma_start(wh1a_sb[:], wh1a_d[:])
            nc.sync.dma_start(wh1b_sb[:], wh1b_d[:])
            for i, g in enumerate(V_GROUPS):
                t0 = vg_t0[i] - 1
                nc.sync.dma_start(vg_sb[i][:], v_d[:, t0 : t0 + g, :, :])

            # fp32 bias bytes for half h live in blob row h cols 642:644
            bias_ap = [
                blob_sb[:, h, WP + W0 : WP + W0 + 2].bitcast(F32)
                for h in range(N_HALF)
            ]

            def v_ap(ty, nty, p, kx):
                """moving operand [128, nty, W] for ty..ty+nty-1, stream p"""
                if ty == 0:
                    assert nty == 1
                    return blob_sb[:, p, kx : kx + W]
                for i in reversed(range(len(V_GROUPS))):
                    if ty >= vg_t0[i]:
                        t = ty - vg_t0[i]
                        assert t + nty <= V_GROUPS[i]
                        return vg_sb[i][:, t : t + nty, p, kx : kx + W]
                raise AssertionError(ty)

            def w_ap(h, p, kx):
                if h == 0:
                    return blob_sb[:, p, WP + kx * 128 : WP + kx * 128 + 128]
                wsb, row = (
                    (wh1a_sb, 0) if p == 1 else (wh1b_sb, WH0_IDX[p] - 1)
                )
                return wsb[:, row, kx * 128 : kx * 128 + 128]

            def bcast2(ap):
                """[128, N] AP -> [128, 2, N] with a stride-0 middle dim"""
                return bass.AP(ap.tensor, ap.offset, [ap.ap[0], [0, 2], ap.ap[1]])

            add = mybir.AluOpType.add
            ident = mybir.ActivationFunctionType.Identity
            UT0 = [sum(UNITS[:i]) for i in range(len(UNITS))]
            sched = [(u, h) for u in range(len(UNITS)) for h in range(2)]
            for u, h in sched:
                nty = UNITS[u]
                ty0 = UT0[u]
                n = nty * W
                if u == len(UNITS) - 1:
                    # last unit: accumulate each output row COMPLETELY in
                    # PSUM (9 MMs per row, odd rows via negated-m2 weights)
                    # so the tail chain is just ACT + DMA - no VectorE drain
                    pl = ppool.tile([128, 2, n], F32, tag="ps", name="ps")
                    for j, plist in ((0, (1, 2, 0)), (1, (1, 5, 3))):
                        for idx, p in enumerate(plist):
                            for kx in range(KW):
                                if p == 5:
                                    lhsT = wn_sb[:, h, kx * 128 : kx * 128 + 128]
                                    vp = 2
                                else:
                                    lhsT = w_ap(h, p, kx)
                                    vp = p
                                nc.tensor.matmul(
                                    pl[:, j, :],
                                    lhsT,
                                    v_ap(ty0, nty, vp, kx),
                                    start=(idx == 0 and kx == 0),
                                    stop=(idx == 2 and kx == KW - 1),
                                )
                    # even rows on ACT, odd on V (parallel tail chains);
                    # each row DMAs out the moment its plane is done
                    oh = opool.tile([128, 2, n], F16, tag="ot", name="ot")
                    nc.scalar.activation(
                        oh[:, 0, :],
                        pl[:, 0, :],
                        ident,
                        bias=bias_ap[h][:, 0:1],
                        scale=1.0,
                    )
                    nc.vector.tensor_scalar(
                        out=oh[:, 1, :],
                        in0=pl[:, 1, :],
                        scalar1=1.0,
                        scalar2=bias_ap[h][:, 0:1],
                        op0=mybir.AluOpType.mult,
                        op1=add,
                    )
                    for j in range(2):
                        # h0's rows on the scalar queue, h1 (critical) on
                        # sync: the tail wire drains on both in parallel
                        eng = nc.scalar if h == 0 else nc.sync
                        eng.dma_start(
                            o_d[:, h, j, ty0 : ty0 + nty, :], oh[:, j, :]
                        )
                    continue
                # plane pairs: ta = [m1 | m2], tb = [m0 | m3']
                ta = ppool.tile([128, 2, n], F32, tag="ps", name="ps")
                tb = ppool.tile([128, 2, n], F32, tag="ps", name="ps")
                for mt, planes in ((ta, (1, 2)), (tb, (0, 3))):
                    for j, p in enumerate(planes):
                        for kx in range(KW):
                            nc.tensor.matmul(
                                mt[:, j, :],
                                w_ap(h, p, kx),
                                v_ap(ty0, nty, p, kx),
                                start=(kx == 0),
                                stop=(kx == KW - 1),
                            )
                # ScalarE: stage m1+bias and +-m2 as fp16
                s1 = spool.tile([128, n], F16, tag="s1", name="s1")
                s2 = spool.tile([128, 2, n], F16, tag="s2", name="s2")
                nc.scalar.activation(
                    s1[:],
                    ta[:, 0, :],
                    ident,
                    bias=bias_ap[h][:, 0:1],
                    scale=1.0,
                )
                nc.scalar.activation(
                    s2[:, 0, :], ta[:, 1, :], ident, bias=0.0, scale=1.0
                )
                nc.scalar.activation(
                    s2[:, 1, :], ta[:, 1, :], ident, bias=0.0, scale=-1.0
                )
                # VectorE: X = [m0|m3'] + [s1|s1] (stride-0 broadcast),
                # OUT = X + [s2|-s2] -> [even|odd] rows contiguous
                xp = spool.tile([128, 2, n], F16, tag="xp", name="xp")
                nc.vector.scalar_tensor_tensor(
                    out=xp[:],
                    in0=tb[:],
                    scalar=0.0,
                    in1=bcast2(s1[:]),
                    op0=add,
                    op1=add,
                )
                oh = opool.tile([128, 2, n], F16, tag="ot", name="ot")
                nc.vector.scalar_tensor_tensor(
                    out=oh[:],
                    in0=xp[:],
                    scalar=0.0,
                    in1=s2[:],
                    op0=add,
                    op1=add,
                )
                eng = (
                    nc.scalar if (u == len(UNITS) - 2 and h == 1) else nc.sync
                )
                eng.dma_start(o_d[:, h, :, ty0 : ty0 + nty, :], oh[:])
    nc.compile()
    return nc


def kernel(x, weight, bias):
    global LAST_RESULT
    if "nc" not in _NC_CACHE:
        _NC_CACHE["nc"] = _build()
    nc = _NC_CACHE["nc"]

    x = np.ascontiguousarray(np.asarray(x, dtype=np.float32))
    weight = np.asarray(weight, dtype=np.float32)
    bias = np.asarray(bias, dtype=np.float32)

    # zero-padded image; host computes the y-direction Winograd transform
    xpad = np.zeros((C_IN, H + 2, WP), dtype=np.float32)
    xpad[:, 1 : H + 1, 1 : W + 1] = x
    TY = H // 2
    r = 2 * np.arange(TY)
    v_full = np.empty((4, C_IN, TY, WP), dtype=np.float16)
    v_full[0] = xpad[:, r] - xpad[:, r + 2]
    v_full[1] = xpad[:, r + 1] + xpad[:, r + 2]
    v_full[2] = xpad[:, r + 2] - xpad[:, r + 1]
    v_full[3] = xpad[:, r + 1] - xpad[:, r + 3]

    # winograd weights per (p, kx): [c, o] fp16; w~3 NEGATED (odd row adds)
    wT = weight.transpose(1, 2, 3, 0)  # [c, ky, kx, o]
    wt = np.empty((4, KW, C_IN, C_OUT), dtype=np.float16)
    wt[0] = wT[:, 0].transpose(1, 0, 2)
    wt[1] = ((wT[:, 0] + wT[:, 1] + wT[:, 2]) / 2).transpose(1, 0, 2)
    wt[2] = ((wT[:, 0] - wT[:, 1] + wT[:, 2]) / 2).transpose(1, 0, 2)
    wt[3] = (-wT[:, 2]).transpose(1, 0, 2)

    # b[p, h] = bias[h*128 + p] in fp32, as raw bytes in fp16-typed cols
    bh = np.ascontiguousarray(bias.reshape(N_HALF, 128).T.astype(np.float32))
    bhv = bh.view(np.float16)  # [128, 4]

    def wpack(p, o0):
        return wt[p][:, :, o0 : o0 + 128].transpose(1, 0, 2).reshape(C_IN, KW * 128)

    wh1a = wpack(1, 128).reshape(C_IN, 1, W0)
    wh1b = np.stack([wpack(p, 128) for p in (2, 0, 3)], axis=1)
    wn = np.stack([-wpack(2, 0), -wpack(2, 128)], axis=1)

    in_maps = []
    for c in range(N_CORES):
        t0 = c * TY_S
        blob = np.zeros((C_IN, 4, BLOB_C), dtype=np.float16)
        for p in range(4):
            blob[:, p, :WP] = v_full[p][:, t0, :]
            blob[:, p, WP : WP + W0] = wpack(p, 0)
        blob[:, 0, WP + W0 :] = bhv[:, 0:2]
        blob[:, 1, WP + W0 :] = bhv[:, 2:4]
        vc = np.ascontiguousarray(
            v_full[:, :, t0 + 1 : t0 + TY_S, :].transpose(1, 2, 0, 3)
        )
        in_maps.append(
            {
                "vw": blob,
                "wh1a": wh1a,
                "wh1b": wh1b,
                "wn": wn,
                "v": vc,
            }
        )

    kw = dict(TRACE_KW)
    if TRACE:
        kw.setdefault("trace", True)
        kw.setdefault("trace_cores", [0])
    res = bass_utils.run_bass_kernel_spmd(
        nc, in_maps, core_ids=list(range(N_CORES)), **kw
    )
    LAST_RESULT = res

    out = np.empty((C_OUT, H, W), dtype=np.float32)
    for c in range(N_CORES):
        # device layout [p, half, eo, ty, x]: channel h*128+p, row 2*ty+eo
        arr = res.results[c]["out"].astype(np.float32)
        out[:, c * H_S : (c + 1) * H_S, :] = arr.transpose(1, 0, 3, 2, 4).reshape(
            C_OUT, H_S, W
        )
    return out


# revision 62
# speedup vs baseline: 1.1078x; 1.0163x over previous
"""Conv2d(128->256, 3x3, pad=1) over a 256x256 image, sharded across 8 trn2 cores.

Strategy: 1-D Winograd F(2,3) along Y, all-fp16.
---------------------------------------------------
x: (C_in=128, H=256, W=256) f32, weight: (256, 128, 3, 3), bias: (256,1,1).
C_in == 128 maps exactly onto the SBUF partition (contraction) dim.

The 3x3 conv is decomposed as 3 kx taps x a 3-tap FIR in y. The y FIR is
computed with Winograd F(2,3): per output row PAIR (ty), the host builds 4
transformed row streams from padded input rows d_r = xpad[2ty + r]:

    v0 = d0 - d2,  v1 = d1 + d2,  v2 = d2 - d1,  v3 = d1 - d3

and 4 transformed weight sets per kx (g_r = w[:, :, ky=r, kx]):

    w~0 = g0, w~1 = (g0+g1+g2)/2, w~2 = (g0-g1+g2)/2, w~3 = -g2  (NEGATED)

On device, 4 plane matmuls per (sub-unit = 2 ty x one out-channel half),
each accumulating 3 kx taps (N=512):

    m_p = sum_kx  w~_p[kx].T @ v_p[ty0:ty0+2, kx:kx+W]

    out[2ty]   = m0 + m1 + m2 + bias           (even rows)
    out[2ty+1] = m1 - m2 + m3' + bias          (odd rows; m3' = -g2 conv)

That is 12 N=512 matmuls per 2ty-half where direct conv needs 18: a 33%
reduction in PE cycles, with NO fp8 anywhere (the old kernel ran 2 of 9
taps in fp8e4m3 at 1.74e-2 rel err; this is ~7e-4).

The inverse transform must combine PSUM planes post-PE (that sharing of
m1/m2 between even and odd rows IS the Winograd saving) and is budgeted
across the two fused-op engines (measured [128,512] op costs: ACT 687ns,
DVE-with-PSUM-operand 751ns, DVE 16-bit SBUF-only ~480ns):

  ScalarE: s1 = m1 + bias (fp16), s2p = +m2, s2n = -m2      3 ops ~2.0us
  VectorE: X = [m0|m3'] + bcast(s1)    (one op, stride-0 broadcast AP)
           OUT = X + [s2p|s2n] -> fp16 out rows             2 ops ~2.0us

against a 2.6us matmul span per sub-unit. PSUM planes pack pairwise into
[128, 2, 512] tiles ([m0|m3'], [m1|m2]) = 2 banks each; ring of 2
sub-units fills all 8 banks. Walrus rejects scalar_tensor_tensor with two
PSUM operands and ANY gpsimd tensor op, hence this exact split.

The LAST unit instead accumulates each output row COMPLETELY in PSUM
(9 MMs/row, odd rows via a negated-m2 weight copy, +50% PE for that unit)
so its tail chain is one ACT / one V op + tiny row DMAs - the kernel end
does not wait on a VectorE backlog.

Sharding: H split across 8 cores (32 output rows = 16 ty pairs each); the
y halo is absorbed into the host transform (v rows never cross units).

DMA: one sync-engine queue carries all inputs in strict need-order. The
startup mega-blob fuses v_ty0 + ALL half-0 weights + the fp32 bias (raw
bytes in fp16 cols) into ONE transfer: the whole first unit unblocks
atomically, and every later transfer lands >=1 sub-unit ahead of use, so
the early matmul stream is gap-free BY CONSTRUCTION (stochastic wire
jitter otherwise opens 0.3-2us holes that reset the HAM busy-window and
leave the PE at 1.2 GHz for several extra us - measured repeatedly).
10 dep-free dummy matmuls bridge engine-boot (~6.6us) to blob-landing
(~11us), warming the clock gate before real work; a dummy activation
absorbs the one-time ~1.3us ACT_TABLE_LOAD. wn rides the scalar HWDGE
queue at boot (warms queue 10); the final units' output DMAs split across
both queues so the tail wire drains in parallel. Output rows go to DRAM
deinterleaved ([p, half, even/odd, ty, x]); the host re-interleaves
during the gather.
"""

import numpy as np

import concourse.bass as bass
import concourse.tile as tile
from concourse import bacc, mybir
from concourse import bass_utils

N_CORES = 8
C_IN, C_OUT, KH, KW = 128, 256, 3, 3
H, W = 256, 256
H_S = H // N_CORES            # 32 output rows per core
TY_S = H_S // 2               # 16 winograd row-pair units per core
WP = W + 2                    # padded row width: 258
N_HALF = C_OUT // 128         # 2 output-channel halves

F32 = mybir.dt.float32
F16 = mybir.dt.float16

# startup mega-blob row p = [v_p ty0 (258) | h0 weights w~_p[kx0..2] (384) |
# 2 bias cols]: ONE transfer unblocks the whole first unit atomically, so
# the early stream has no per-transfer arrival gaps (HAM window stays busy)
W0 = 3 * 128                  # per-plane weight cols
BLOB_C = WP + W0 + 2          # 644
WH0_IDX = {1: 0, 2: 1, 0: 2, 3: 3}
# units in ty pairs: first two 1-ty (fast start), last two 1-ty (short tail)
UNITS = [1, 1, 2, 2, 2, 2, 2, 2, 1, 1]
assert sum(UNITS) == TY_S
# v DMA groups for ty 1..15 (ty0 rides the blob)
V_GROUPS = [1, 2, 4, 8]
assert sum(V_GROUPS) == TY_S - 1

WARMUP_N512 = 10

# Set by test harness: TRACE=True makes the next kernel() call capture an
# NTFF profile; the BassKernelResults lands in LAST_RESULT.
TRACE = False
TRACE_KW = {}
LAST_RESULT = None

_NC_CACHE = {}


def _build():
    nc = bacc.Bacc(
        "TRN2",
        target_bir_lowering=False,
        debug=False,
        enable_asserts=False,
        num_devices=N_CORES,
    )
    vw_d = nc.dram_tensor("vw", [C_IN, 4, BLOB_C], F16, kind="ExternalInput").ap()
    wh1a_d = nc.dram_tensor("wh1a", [C_IN, 1, W0], F16, kind="ExternalInput").ap()
    wh1b_d = nc.dram_tensor("wh1b", [C_IN, 3, W0], F16, kind="ExternalInput").ap()
    # negated-m2 weights for the last unit's direct-accumulated odd rows
    wn_d = nc.dram_tensor("wn", [C_IN, 2, W0], F16, kind="ExternalInput").ap()
    v_d = nc.dram_tensor(
        "v", [C_IN, TY_S - 1, 4, WP], F16, kind="ExternalInput"
    ).ap()
    # output laid out [p, half, even/odd, ty, x] (channel h*128+p at [p, h];
    # row 2*ty+eo at [eo, ty]); host deinterleaves rows during the gather
    o_d = nc.dram_tensor(
        "out", [128, N_HALF, 2, TY_S, W], F16, kind="ExternalOutput"
    ).ap()

    with tile.TileContext(nc) as tc:
        with (
            tc.tile_pool(name="vin", bufs=1) as vpool,
            tc.tile_pool(name="wts", bufs=1) as wpool,
            tc.tile_pool(name="acc", bufs=4, space="PSUM") as ppool,
            tc.tile_pool(name="scr", bufs=8) as spool,
            tc.tile_pool(name="outs", bufs=6) as opool,
        ):
            # PE warmup: dep-free garbage through the PE into a dead tile
            warm_sb = nc.alloc_sbuf_tensor("warm_src", [128, 512], F16).ap()
            warm_ps = ppool.tile([128, 2, 512], F32, tag="ps", name="ps")
            for _ in range(WARMUP_N512):
                nc.tensor.matmul(warm_ps[:, 0, :], warm_sb[:, :128], warm_sb[:])
            # ACT warmup: absorb the ~1.3us one-time ACT_TABLE_LOAD during
            # boot so the first real finish op isn't delayed by it
            warm_act = nc.alloc_sbuf_tensor("warm_act", [128, 16], F16).ap()
            nc.scalar.activation(
                warm_act[:], warm_sb[:, :16],
                mybir.ActivationFunctionType.Identity, bias=0.0, scale=1.0,
            )

            blob_sb = wpool.tile([128, 4, BLOB_C], F16, tag="vw", name="vw")
            wh1a_sb = wpool.tile([128, 1, W0], F16, tag="wh1a", name="wh1a")
            wh1b_sb = wpool.tile([128, 3, W0], F16, tag="wh1b", name="wh1b")
            vg_sb = [
                vpool.tile([128, g, 4, WP], F16, tag=f"vg{i}", name=f"vg{i}")
                for i, g in enumerate(V_GROUPS)
            ]
            vg_t0 = [1 + sum(V_GROUPS[:i]) for i in range(len(V_GROUPS))]

            # single queue, strict need-order
            wn_sb = wpool.tile([128, 2, W0], F16, tag="wn", name="wn")
            # wn rides the scalar HWDGE queue at boot: it is only needed at
            # the very end, and the transfer warms up queue 10 so the tail
            # output DMAs routed there don't pay its ~2us cold start
            nc.scalar.dma_start(wn_sb[:], wn_d[:])
            # sync queue, strict need-order; sub-units consume in the
            # interleaved order u0h0, u1h0, u0h1, u1h1, u2h0, ... so the h1
            # weights and early v groups have extra time to land
            nc.sync.dma_start(blob_sb[:], vw_d[:])
            nc.sync.dma_start(wh1a_sb[:], wh1a_d[:])
            nc.sync.dma_start(wh1b_sb[:], wh1b_d[:])
            for i, g in enumerate(V_GROUPS):
                t0 = vg_t0[i] - 1
                nc.sync.dma_start(vg_sb[i][:], v_d[:, t0 : t0 + g, :, :])

            # fp32 bias bytes for half h live in blob row h cols 642:644
            bias_ap = [
                blob_sb[:, h, WP + W0 : WP + W0 + 2].bitcast(F32)
                for h in range(N_HALF)
            ]

            def v_ap(ty, nty, p, kx):
                """moving operand [128, nty, W] for ty..ty+nty-1, stream p"""
                if ty == 0:
                    assert nty == 1
                    return blob_sb[:, p, kx : kx + W]
                for i in reversed(range(len(V_GROUPS))):
                    if ty >= vg_t0[i]:
                        t = ty - vg_t0[i]
                        assert t + nty <= V_GROUPS[i]
                        return vg_sb[i][:, t : t + nty, p, kx : kx + W]
                raise AssertionError(ty)

            def w_ap(h, p, kx):
                if h == 0:
                    return blob_sb[:, p, WP + kx * 128 : WP + kx * 128 + 128]
                wsb, row = (
                    (wh1a_sb, 0) if p == 1 else (wh1b_sb, WH0_IDX[p] - 1)
                )
                return wsb[:, row, kx * 128 : kx * 128 + 128]

            def bcast2(ap):
                """[128, N] AP -> [128, 2, N] with a stride-0 middle dim"""
                return bass.AP(ap.tensor, ap.offset, [ap.ap[0], [0, 2], ap.ap[1]])

            add = mybir.AluOpType.add
            ident = mybir.ActivationFunctionType.Identity
            UT0 = [sum(UNITS[:i]) for i in range(len(UNITS))]
            sched = [(u, h) for u in range(len(UNITS)) for h in range(2)]
            for u, h in sched:
                nty = UNITS[u]
                ty0 = UT0[u]
                n = nty * W
                if u == len(UNITS) - 1:
                    # last unit: accumulate each output row COMPLETELY in
                    # PSUM (9 MMs per row, odd rows via negated-m2 weights)
                    # so the tail chain is just ACT + DMA - no VectorE drain
                    pl = ppool.tile([128, 2, n], F32, tag="ps", name="ps")
                    for j, plist in ((0, (1, 2, 0)), (1, (1, 5, 3))):
                        for idx, p in enumerate(plist):
                            for kx in range(KW):
                                if p == 5:
                                    lhsT = wn_sb[:, h, kx * 128 : kx * 128 + 128]
                                    vp = 2
                                else:
                                    lhsT = w_ap(h, p, kx)
                                    vp = p
                                nc.tensor.matmul(
                                    pl[:, j, :],
                                    lhsT,
                                    v_ap(ty0, nty, vp, kx),
                                    start=(idx == 0 and kx == 0),
                                    stop=(idx == 2 and kx == KW - 1),
                                )
                    # even rows on ACT, odd on V (parallel tail chains);
                    # each row DMAs out the moment its plane is done
                    oh = opool.tile([128, 2, n], F16, tag="ot", name="ot")
                    nc.scalar.activation(
                        oh[:, 0, :],
                        pl[:, 0, :],
                        ident,
                        bias=bias_ap[h][:, 0:1],
                        scale=1.0,
                    )
                    nc.vector.tensor_scalar(
                        out=oh[:, 1, :],
                        in0=pl[:, 1, :],
                        scalar1=1.0,
                        scalar2=bias_ap[h][:, 0:1],
                        op0=mybir.AluOpType.mult,
                        op1=add,
                    )
                    for j in range(2):
                        # h0's rows on the scalar queue, h1 (critical) on
                        # sync: the tail wire drains on both in parallel
                        eng = nc.scalar if h == 0 else nc.sync
                        eng.dma_start(
                            o_d[:, h, j, ty0 : ty0 + nty, :], oh[:, j, :]
                        )
                    continue
                # plane pairs: ta = [m1 | m2], tb = [m0 | m3']
                ta = ppool.tile([128, 2, n], F32, tag="ps", name="ps")
                tb = ppool.tile([128, 2, n], F32, tag="ps", name="ps")
                for mt, planes in ((ta, (1, 2)), (tb, (0, 3))):
                    for j, p in enumerate(planes):
                        for kx in range(KW):
                            nc.tensor.matmul(
                                mt[:, j, :],
                                w_ap(h, p, kx),
                                v_ap(ty0, nty, p, kx),
                                start=(kx == 0),
                                stop=(kx == KW - 1),
                            )
                # ScalarE: stage m1+bias and +-m2 as fp16
                s1 = spool.tile([128, n], F16, tag="s1", name="s1")
                s2 = spool.tile([128, 2, n], F16, tag="s2", name="s2")
                nc.scalar.activation(
                    s1[:],
                    ta[:, 0, :],
                    ident,
                    bias=bias_ap[h][:, 0:1],
                    scale=1.0,
                )
                nc.scalar.activation(
                    s2[:, 0, :], ta[:, 1, :], ident, bias=0.0, scale=1.0
                )
                nc.scalar.activation(
                    s2[:, 1, :], ta[:, 1, :], ident, bias=0.0, scale=-1.0
                )
                # VectorE: X = [m0|m3'] + [s1|s1] (stride-0 broadcast),
                # OUT = X + [s2|-s2] -> [even|odd] rows contiguous
                xp = spool.tile([128, 2, n], F16, tag="xp", name="xp")
                nc.vector.scalar_tensor_tensor(
                    out=xp[:],
                    in0=tb[:],
                    scalar=0.0,
                    in1=bcast2(s1[:]),
                    op0=add,
                    op1=add,
                )
                oh = opool.tile([128, 2, n], F16, tag="ot", name="ot")
                nc.vector.scalar_tensor_tensor(
                    out=oh[:],
                    in0=xp[:],
                    scalar=0.0,
                    in1=s2[:],
                    op0=add,
                    op1=add,
                )
                eng = (
                    nc.scalar if (u == len(UNITS) - 2 and h == 1) else nc.sync
                )
                eng.dma_start(o_d[:, h, :, ty0 : ty0 + nty, :], oh[:])
    nc.compile()
    return nc


def kernel(x, weight, bias):
    global LAST_RESULT
    if "nc" not in _NC_CACHE:
        _NC_CACHE["nc"] = _build()
    nc = _NC_CACHE["nc"]

    x = np.ascontiguousarray(np.asarray(x, dtype=np.float32))
    weight = np.asarray(weight, dtype=np.float32)
    bias = np.asarray(bias, dtype=np.float32)

    # zero-padded image; host computes the y-direction Winograd transform
    xpad = np.zeros((C_IN, H + 2, WP), dtype=np.float32)
    xpad[:, 1 : H + 1, 1 : W + 1] = x
    TY = H // 2
    r = 2 * np.arange(TY)
    v_full = np.empty((4, C_IN, TY, WP), dtype=np.float16)
    v_full[0] = xpad[:, r] - xpad[:, r + 2]
    v_full[1] = xpad[:, r + 1] + xpad[:, r + 2]
    v_full[2] = xpad[:, r + 2] - xpad[:, r + 1]
    v_full[3] = xpad[:, r + 1] - xpad[:, r + 3]

    # winograd weights per (p, kx): [c, o] fp16; w~3 NEGATED (odd row adds)
    wT = weight.transpose(1, 2, 3, 0)  # [c, ky, kx, o]
    wt = np.empty((4, KW, C_IN, C_OUT), dtype=np.float16)
    wt[0] = wT[:, 0].transpose(1, 0, 2)
    wt[1] = ((wT[:, 0] + wT[:, 1] + wT[:, 2]) / 2).transpose(1, 0, 2)
    wt[2] = ((wT[:, 0] - wT[:, 1] + wT[:, 2]) / 2).transpose(1, 0, 2)
    wt[3] = (-wT[:, 2]).transpose(1, 0, 2)

    # b[p, h] = bias[h*128 + p] in fp32, as raw bytes in fp16-typed cols
    bh = np.ascontiguousarray(bias.reshape(N_HALF, 128).T.astype(np.float32))
    bhv = bh.view(np.float16)  # [128, 4]

    def wpack(p, o0):
        return wt[p][:, :, o0 : o0 + 128].transpose(1, 0, 2).reshape(C_IN, KW * 128)

    wh1a = wpack(1, 128).reshape(C_IN, 1, W0)
    wh1b = np.stack([wpack(p, 128) for p in (2, 0, 3)], axis=1)
    wn = np.stack([-wpack(2, 0), -wpack(2, 128)], axis=1)

    in_maps = []
    for c in range(N_CORES):
        t0 = c * TY_S
        blob = np.zeros((C_IN, 4, BLOB_C), dtype=np.float16)
        for p in range(4):
            blob[:, p, :WP] = v_full[p][:, t0, :]
            blob[:, p, WP : WP + W0] = wpack(p, 0)
        blob[:, 0, WP + W0 :] = bhv[:, 0:2]
        blob[:, 1, WP + W0 :] = bhv[:, 2:4]
        vc = np.ascontiguousarray(
            v_full[:, :, t0 + 1 : t0 + TY_S, :].transpose(1, 2, 0, 3)
        )
        in_maps.append(
            {
                "vw": blob,
                "wh1a": wh1a,
                "wh1b": wh1b,
                "wn": wn,
                "v": vc,
            }
        )

    kw = dict(TRACE_KW)
    if TRACE:
        kw.setdefault("trace", True)
        kw.setdefault("trace_cores", [0])
    res = bass_utils.run_bass_kernel_spmd(
        nc, in_maps, core_ids=list(range(N_CORES)), **kw
    )
    LAST_RESULT = res

    out = np.empty((C_OUT, H, W), dtype=np.float32)
    for c in range(N_CORES):
        # device layout [p, half, eo, ty, x]: channel h*128+p, row 2*ty+eo
        arr = res.results[c]["out"].astype(np.float32)
        out[:, c * H_S : (c + 1) * H_S, :] = arr.transpose(1, 0, 3, 2, 4).reshape(
            C_OUT, H_S, W
        )
    return out
